# revision 1
# baseline (speedup 1.0000x reference)
"""Multi-head attention TRN2 kernel, sharded over 8 NeuronCores.

Sharding: (batch, head-group) — core c handles batch c//4 and heads
(c%4)*4 .. (c%4)*4+3. Each core computes its 4 heads' attention plus its
partial output projection; the host sums the 4 partials per batch and adds bo.

Device-side layout (per core):
  QT, KT: [hd=256, S] (bf16)   -- projections computed transposed
  V:      [S, 4 heads, 65]     -- 64 hd cols + ones col (softmax denominator)
  scores computed transposed [j, i] so the softmax sum and attn@V both
  contract over j (the partition dim) with no transposes of the big matrices.
  relative_pos_bias arrives pre-sliced per head in [h, j, i] layout (bf16)
  with the attention mask folded in as an additive -30000 (host-side prep).
  Matmuls run in bf16 (single-pass on the PE); all accumulation is fp32 in
  PSUM. Set PRECISE=1 to switch the matmul path to f32r (2-pass, ~2x slower,
  rel err ~3e-4 instead of ~4e-3).
"""
import os
import sys

if "/opt/trn_rl_repo" not in sys.path:
    sys.path.insert(0, "/opt/trn_rl_repo")

from contextlib import ExitStack

import ml_dtypes
import numpy as np

B, S, D, H = 2, 2048, 1024, 16
HD = D // H          # 64
NCORES = 8
HPC = 4              # heads per core
HDC = HPC * HD       # 256 head-dim cols per core
P = 128
ISLAB = 512          # i-columns per score slab
NJT = S // P         # 16 j tiles
NISLAB = S // ISLAB  # 4 i slabs
NSC = S // ISLAB     # 4 seq chunks in stage 0
MASK_NEG = np.float32(-200.0)

PRECISE = os.environ.get("PRECISE", "0") == "1"
BIAS8 = os.environ.get("BIAS8", "0") == "1" and not PRECISE
PEB = os.environ.get("PEB", "jt2")

_CACHE = {}


def _build():
    import concourse.bass as bass
    import concourse.mybir as mybir
    import concourse.tile as tile
    from concourse.tile import add_dep_helper
    from concourse import bacc
    from concourse.masks import make_identity

    f32 = mybir.dt.float32
    mmdt = mybir.dt.float32r if PRECISE else mybir.dt.bfloat16
    f8 = mybir.dt.float8e4
    biasdt = f8 if BIAS8 else mmdt
    DR = mybir.MatmulPerfMode.DoubleRow

    # All our ACT funcs (Exp, Ln, Copy, Identity) live together in the
    # 'natural_log_exp_and_others' table set; restricting the registry to it
    # makes insert_act_table_loads emit ONE load instead of thrashing
    # exp<->ln tables at every softmax-normalization point.
    import concourse.hw_specs as hw_specs
    if not getattr(hw_specs, "_mha_table_patch", False):
        _orig_gat = hw_specs.get_activation_tables

        def _one_table(arch, _orig=_orig_gat):
            t = _orig(arch)
            name = "natural_log_exp_and_others"
            if name not in t:
                return t
            keep = t[name]
            # preserve dict size/order (set index == act_func_set_id); just
            # make the shared funcs resolvable only via the ln+exp set
            return {
                k: (v if k == name else (v - keep))
                for k, v in t.items()
            }

        hw_specs.get_activation_tables = _one_table
        bacc.get_activation_tables = _one_table
        hw_specs._mha_table_patch = True

    nc = bacc.Bacc(None, target_bir_lowering=False)

    q_in = nc.declare_dram_parameter("q_in", [S, D], mmdt, isOutput=False)
    k_in = nc.declare_dram_parameter("k_in", [S, D], mmdt, isOutput=False)
    v_in = nc.declare_dram_parameter("v_in", [S, D], mmdt, isOutput=False)
    wq = nc.declare_dram_parameter("wq", [D, HDC], mmdt, isOutput=False)
    wk = nc.declare_dram_parameter("wk", [D, HDC], mmdt, isOutput=False)
    wv = nc.declare_dram_parameter("wv", [D, HDC], mmdt, isOutput=False)
    wo = nc.declare_dram_parameter("wo", [HDC, D], mmdt, isOutput=False)
    bq = nc.declare_dram_parameter("bq", [HDC], f32, isOutput=False)
    bk = nc.declare_dram_parameter("bk", [HDC], f32, isOutput=False)
    bv_rep = nc.declare_dram_parameter("bv_rep", [P, HDC], f32, isOutput=False)
    ones64 = nc.declare_dram_parameter("ones64", [1, 64], mmdt, isOutput=False)
    if BIAS8:
        bias_c = nc.declare_dram_parameter(
            "bias_c", [HPC, NISLAB, 64, NJT, 2, ISLAB], biasdt, isOutput=False)
        ident8 = nc.declare_dram_parameter(
            "ident8", [64, 2, P], biasdt, isOutput=False)
    else:
        bias_c = nc.declare_dram_parameter(
            "bias_c", [HPC, NISLAB, NJT, P, ISLAB], biasdt, isOutput=False)
    out_p = nc.declare_dram_parameter("out_p", [S, D], f32, isOutput=True)

    EXP = mybir.ActivationFunctionType.Exp
    LN = mybir.ActivationFunctionType.Ln

    with tile.TileContext(nc) as tc, ExitStack() as big:
        consts = big.enter_context(tc.tile_pool(name="consts", bufs=1))
        persist = big.enter_context(tc.tile_pool(name="persist", bufs=1))

        ident = consts.tile([P, P], mmdt)
        make_identity(nc, ident)
        ones64_sb = consts.tile([1, 64], mmdt)
        nc.sync.dma_start(ones64_sb, ones64[:])
        bqv = consts.tile([P, 2], f32)
        nc.sync.dma_start(bqv, bq[:].rearrange("(o p) -> p o", p=P))
        bkv = consts.tile([P, 2], f32)
        nc.sync.dma_start(bkv, bk[:].rearrange("(o p) -> p o", p=P))
        bv_sb = consts.tile([P, HDC], f32)
        nc.sync.dma_start(bv_sb, bv_rep[:])
        ones_col = consts.tile([P, 1], f32)
        nc.vector.memset(ones_col, 1.0)
        if BIAS8:
            ident8_sb = consts.tile([64, 2, P], biasdt)
            nc.sync.dma_start(ident8_sb, ident8[:])

        wq_sb = consts.tile([P, 8, HDC], mmdt)
        wk_sb = consts.tile([P, 8, HDC], mmdt)
        wv_sb = consts.tile([P, 8, HDC], mmdt)
        wo_sb = consts.tile([P, 2, D], mmdt)

        def load_weights():
            # deferred so the first input tiles win the DMA queues at startup
            nc.sync.dma_start(wq_sb, wq[:].rearrange("(dk p) m -> p dk m", p=P))
            nc.sync.dma_start(wk_sb, wk[:].rearrange("(dk p) m -> p dk m", p=P))
            nc.sync.dma_start(wv_sb, wv[:].rearrange("(dk p) m -> p dk m", p=P))
            nc.sync.dma_start(wo_sb, wo[:].rearrange("(kt p) n -> p kt n", p=P))

        qt_full = persist.tile([64, HPC, S], mmdt)   # [hd%64, head, seq]
        kt_full = persist.tile([64, HPC, S], mmdt)
        v_full = persist.tile([P, NJT, HPC, HD + 1], mmdt)  # [seq%128, jt, h, hd|1]
        ctxT = persist.tile([P, 2, S], mmdt)      # [hd%128, hd//128, i]

        # ones column of V (softmax denominator trick)
        for jt in range(NJT):
            nc.vector.tensor_copy(
                v_full[:, jt, :, HD:HD + 1],
                ones_col[:, None, :].to_broadcast((P, HPC, 1)))

        # ---------------- Stage 0: transpose inputs + projections ----------
        s0 = big.enter_context(tc.tile_pool(name="s0", bufs=6))
        s0x = big.enter_context(tc.tile_pool(name="s0x", bufs=2))
        with ExitStack() as st0:
            tps = st0.enter_context(tc.tile_pool(name="tps", bufs=3, space="PSUM"))
            pps = st0.enter_context(tc.tile_pool(name="pps", bufs=2, space="PSUM"))
            vps = st0.enter_context(tc.tile_pool(name="vps", bufs=2, space="PSUM"))

            first = True
            for x_dram, which in ((q_in, "q"), (k_in, "k"), (v_in, "v")):
                for sc in range(NSC):
                    xn = []
                    for st in range(4):
                        t = s0.tile([P, D], mmdt, tag="xn")
                        nc.sync.dma_start(t, x_dram[sc * ISLAB + st * P:
                                                    sc * ISLAB + (st + 1) * P, :])
                        xn.append(t)
                    if first:
                        load_weights()
                        first = False
                    xT = s0x.tile([P, 8, ISLAB], mmdt, tag="xT")
                    for dk in range(8):
                        tp4 = tps.tile([P, ISLAB], mmdt, tag="tp4")
                        for st in range(4):
                            nc.tensor.transpose(
                                tp4[:, st * P:(st + 1) * P],
                                xn[st][:, dk * P:(dk + 1) * P], ident)
                        if dk % 2 == 0:
                            nc.scalar.copy(xT[:, dk, :], tp4)
                        else:
                            nc.vector.tensor_copy(xT[:, dk, :], tp4)
                    if which == "v":
                        for st in range(4):
                            vp = vps.tile([P, HDC], f32, tag="vp")
                            for dk in range(8):
                                nc.tensor.matmul(
                                    vp, xT[:, dk, st * P:(st + 1) * P],
                                    wv_sb[:, dk, :],
                                    start=(dk == 0), stop=(dk == 7),
                                )
                            jt = sc * 4 + st
                            nc.vector.tensor_add(
                                v_full[:, jt, :, :HD],
                                vp.rearrange("p (h d) -> p h d", h=HPC),
                                bv_sb.rearrange("p (h d) -> p h d", h=HPC),
                            )
                    else:
                        dst = qt_full if which == "q" else kt_full
                        w_sb = wq_sb if which == "q" else wk_sb
                        bvec = bqv if which == "q" else bkv
                        for mt in range(2):
                            pp = pps.tile([P, ISLAB], f32, tag="pp")
                            for dk in range(8):
                                nc.tensor.matmul(
                                    pp, w_sb[:, dk, mt * P:(mt + 1) * P],
                                    xT[:, dk, :],
                                    start=(dk == 0), stop=(dk == 7),
                                )
                            sl = slice(sc * ISLAB, (sc + 1) * ISLAB)
                            nc.vector.tensor_scalar_add(
                                dst[:, 2 * mt, sl], pp[0:64],
                                bvec[0:64, mt:mt + 1],
                            )
                            nc.vector.tensor_scalar_add(
                                dst[:, 2 * mt + 1, sl], pp[64:128],
                                bvec[64:128, mt:mt + 1],
                            )

        # ---------------- Stage 1: attention ------------------------------
        sbias = big.enter_context(tc.tile_pool(name="sbias", bufs=3))
        sein = big.enter_context(tc.tile_pool(name="sein", bufs=6))
        sexp = big.enter_context(tc.tile_pool(name="sexp", bufs=8))
        snrm = big.enter_context(tc.tile_pool(name="snrm", bufs=2))
        dnrm = big.enter_context(tc.tile_pool(name="dnrm", bufs=2, space="DRAM"))
        so = big.enter_context(tc.tile_pool(name="so", bufs=3))
        with ExitStack() as st1:
            sps = st1.enter_context(tc.tile_pool(name="sps", bufs=5, space="PSUM"))
            ops2 = st1.enter_context(tc.tile_pool(name="ops2", bufs=1, space="PSUM"))
            cps = st1.enter_context(tc.tile_pool(name="cps", bufs=2, space="PSUM"))

            LAG = 5

            def make_norm_steps(cp, hp, ho, isl):
                # Deferred softmax normalization for one finished slab:
                # 1/denom via exp(-ln(d)) on ACT, partition-broadcast via a
                # DRAM bounce, multiply on DVE. Emitted as discrete steps that
                # the caller interleaves into the NEXT slab's loop so these
                # never stall the in-order ACT/PE streams at slab boundaries.
                state = {}

                def s_ln():
                    state["lns"] = snrm.tile([1, ISLAB], f32, tag="lns", name="lns")
                    nc.scalar.activation(state["lns"], cp[HD:HD + 1, :], LN)

                def s_recip():
                    state["recip"] = snrm.tile([1, ISLAB], f32, tag="recip", name="recip")
                    nc.scalar.activation(state["recip"], state["lns"], EXP,
                                         scale=-1.0)

                def s_dma1():
                    state["dns"] = dnrm.tile([1, ISLAB], f32, tag="dns", name="dns")
                    nc.sync.dma_start(state["dns"], state["recip"])

                def s_dma2():
                    dns = state["dns"]
                    state["bsb"] = snrm.tile([64, ISLAB], f32, tag="bsb", name="bsb")
                    nc.sync.dma_start(state["bsb"], bass.AP(
                        tensor=dns.tensor, offset=dns.offset,
                        ap=[[0, 64]] + list(dns[0].ap)))

                def s_mul():
                    nc.vector.tensor_mul(
                        ctxT[hp:hp + 64, ho, isl * ISLAB:(isl + 1) * ISLAB],
                        cp[:HD, :], state["bsb"],
                    )

                return [s_ln, s_recip, s_dma1, s_dma2, s_mul]

            pending = []
            NORM_AT = {3: 0, 6: 1, 9: 2, 12: 3, 15: 4}
            pending_out = []

            def make_outproj(isl):
                # output projection for one finished i-slab (all 4 heads
                # normalized); emitted between later slabs to keep the tail
                # short and the PE stream dense.
                def run():
                    for it in range(isl * 4, isl * 4 + 4):
                        for nt in range(2):
                            op = ops2.tile([P, ISLAB], f32, tag="op", name="op")
                            for kt in range(2):
                                nc.tensor.matmul(
                                    op, ctxT[:, kt, it * P:(it + 1) * P],
                                    wo_sb[:, kt, nt * ISLAB:(nt + 1) * ISLAB],
                                    start=(kt == 0), stop=(kt == 1),
                                )
                            ot = so.tile([P, ISLAB], f32, tag="ot", name="ot")
                            nc.scalar.copy(ot, op)
                            nc.sync.dma_start(
                                out_p[it * P:(it + 1) * P,
                                      nt * ISLAB:(nt + 1) * ISLAB], ot)
                return run

            for isl in range(NISLAB):
                for h in range(HPC):
                    pe_bias = None  # per-jt below
                    hp = (h % 2) * 64   # base partition of this head in qt/kt
                    ho = h // 2         # outer index
                    qt_h = qt_full[:, h, isl * ISLAB:(isl + 1) * ISLAB]
                    cp = cps.tile([HD + 1, ISLAB], f32, tag="cp")
                    ets = [None] * NJT
                    sc_insts = [None] * NJT
                    for jt in range(NJT + LAG):
                        if jt in NORM_AT and pending:
                            pending[NORM_AT[jt]]()
                        if jt < NJT:
                            sp = sps.tile([P, ISLAB], f32, tag="sp")
                            pe_bias = True if (BIAS8 or PEB == "all") else (jt < 2)
                            smm = nc.tensor.matmul(
                                sp, kt_full[:, h, jt * P:(jt + 1) * P],
                                qt_h, start=True, stop=not pe_bias,
                            )
                            sc_insts[jt] = smm
                            if BIAS8:
                                if jt % 4 == 0:
                                    bt4 = sbias.tile([64, 4, 2, ISLAB], biasdt,
                                                     tag="bt", name="bt")
                                    nc.sync.dma_start(
                                        bt4,
                                        bias_c[h, isl, :, jt // 4 * 4:
                                               jt // 4 * 4 + 4])
                                bt = bt4[:, jt % 4]
                            else:
                                if jt % 4 == 0:
                                    bt4 = sbias.tile([P, 4, ISLAB], biasdt,
                                                     tag="bt", name="bt")
                                    nc.sync.dma_start(
                                        bt4,
                                        bias_c[h, isl, jt:jt + 4].rearrange(
                                            "j p c -> p j c"))
                                bt = bt4[:, jt % 4, :]
                            et = sexp.tile([P, ISLAB], mmdt, tag="et")
                            if pe_bias:
                                # fold bias into PSUM on the PE: keeps the PE
                                # stream dense so the HAM clock-gate opens
                                if BIAS8:
                                    nc.tensor.matmul(sp, ident8_sb, bt,
                                                     start=False, stop=True,
                                                     perf_mode=DR)
                                else:
                                    nc.tensor.matmul(sp, ident, bt,
                                                     start=False, stop=True)
                                nc.scalar.activation(et, sp, EXP)
                            else:
                                ein = sein.tile([P, ISLAB], f32, tag="ein")
                                nc.vector.tensor_add(ein, sp, bt)
                                nc.scalar.activation(et, ein, EXP)
                            ets[jt] = et
                        if jt >= LAG:
                            j2 = jt - LAG
                            cmm = nc.tensor.matmul(
                                cp, v_full[:, j2, h, :], ets[j2],
                                start=(j2 == 0), stop=(j2 == NJT - 1),
                            )
                            if jt < NJT:
                                # keep the software-pipeline skew in the PE
                                # stream: ctx(j2) goes AFTER scores(j2+LAG)
                                add_dep_helper(
                                    sc_insts[jt].ins, cmm.ins, sync=False,
                                    reason="preserve scores/ctx LAG skew")
                    pending = make_norm_steps(cp, hp, ho, isl)
                    if h == 1 and isl > 0:
                        # previous i-slab fully normalized once our h==0
                        # slab's interleaved norm steps ran; project it now
                        pending_out[-1]()
                        pending_out.pop()
                if isl > 0:
                    pass
                pending_out.append(make_outproj(isl))
            for step in pending:
                step()
            for run in pending_out:
                run()


    nc.compile()
    return nc


def _get_nc():
    if "nc" not in _CACHE:
        _CACHE["nc"] = _build()
    return _CACHE["nc"]


def _pack_bias(bias_hji, biasdt):
    # bias_hji: [HPC, S(j), S(i)] float32
    if BIAS8:
        # [h, isl, ki, jt, p, ic]: j = jt*128 + ki*2 + p, i = isl*512 + ic
        a = bias_hji.reshape(HPC, NJT, 64, 2, NISLAB, ISLAB)
        a = a.transpose(0, 4, 2, 1, 3, 5)
        return np.ascontiguousarray(a).astype(biasdt)
    a = bias_hji.reshape(HPC, NJT, P, NISLAB, ISLAB).transpose(0, 3, 1, 2, 4)
    return np.ascontiguousarray(a).astype(biasdt)


def _ident8():
    ki, p, m = np.meshgrid(np.arange(64), np.arange(2), np.arange(P),
                           indexing="ij")
    return (m == 2 * ki + p).astype(ml_dtypes.float8_e4m3)


def _prep_inputs(query, key, value, mask, relative_pos_bias,
                 Wq, bq, Wk, bk, Wv, bv, Wo, bo):
    f32 = np.float32
    mmdt = f32 if PRECISE else ml_dtypes.bfloat16
    biasdt = ml_dtypes.float8_e4m3 if BIAS8 else mmdt
    query = np.asarray(query, f32)
    key = np.asarray(key, f32)
    value = np.asarray(value, f32)
    rpb_T = np.ascontiguousarray(
        np.asarray(relative_pos_bias, f32).transpose(2, 0, 1))  # [H, S(j), S(i)]
    # additive mask in [j, i] orientation per batch
    mask_ji = np.asarray(mask)[:, 0].transpose(0, 2, 1)
    madd = np.where(mask_ji == 0, MASK_NEG, f32(0.0)).astype(f32)

    scale = f32(1.0 / np.sqrt(HD))
    Wq_s = (np.asarray(Wq, f32) * scale)
    bq_s = (np.asarray(bq, f32) * scale)
    Wk = np.asarray(Wk, f32)
    Wv = np.asarray(Wv, f32)
    Wo = np.asarray(Wo, f32)
    bk = np.asarray(bk, f32)
    bv = np.asarray(bv, f32)
    ones64 = np.ones((1, 64), mmdt)

    in_maps = []
    for c in range(NCORES):
        b = c // 4
        h0 = (c % 4) * HPC
        cols = slice(h0 * HD, (h0 + HPC) * HD)
        in_maps.append({
            "q_in": query[b].astype(mmdt),
            "k_in": key[b].astype(mmdt),
            "v_in": value[b].astype(mmdt),
            "wq": np.ascontiguousarray(Wq_s[:, cols]).astype(mmdt),
            "wk": np.ascontiguousarray(Wk[:, cols]).astype(mmdt),
            "wv": np.ascontiguousarray(Wv[:, cols]).astype(mmdt),
            "wo": np.ascontiguousarray(Wo[cols, :]).astype(mmdt),
            "bq": np.ascontiguousarray(bq_s[cols]),
            "bk": np.ascontiguousarray(bk[cols]),
            "bv_rep": np.ascontiguousarray(
                np.broadcast_to(bv[cols], (P, HDC))),
            "ones64": ones64,
            **({"ident8": _ident8()} if BIAS8 else {}),
            "bias_c": _pack_bias(rpb_T[h0:h0 + HPC] + madd[b][None], biasdt),
        })
    return in_maps


def run_sharded(run_kwargs=None, **inputs):
    """Build + run on 8 cores; returns (output, BassKernelResults)."""
    from concourse.bass_utils import run_bass_kernel_spmd

    nc = _get_nc()
    in_maps = _prep_inputs(**inputs)
    res = run_bass_kernel_spmd(nc, in_maps, list(range(NCORES)),
                               **(run_kwargs or {}))
    bo = np.asarray(inputs["bo"], np.float32)
    out = np.zeros((B, S, D), np.float32)
    for c in range(NCORES):
        out[c // 4] += res.results[c]["out_p"]
    out += bo[None, None, :]
    return out, res


def kernel(**inputs):
    out, _ = run_sharded(**inputs)
    return out



# revision 14
# speedup vs baseline: 1.1565x; 1.1565x over previous
"""Multi-head attention TRN2 kernel, sharded over 8 NeuronCores.

Sharding: (batch, head-group) — core c handles batch c//4 and heads
(c%4)*4 .. (c%4)*4+3. Each core computes its 4 heads' attention plus its
partial output projection; the host sums the 4 partials per batch and adds bo.

v2 layout (vs the earlier transpose-on-device version):
  - q/k/v arrive HOST-pre-transposed as xT [8, 128, S] (d-major) so stage 0
    is pure projection matmuls (no PE transposes, no PSUM->SBUF copy pairs);
    q/k/v biases are folded in as K=1 ones-row matmuls.
  - K/Q projections are head-PAIR packed: kt2/qt2 [128, pair, S] hold head
    2p on partitions 0-63 and head 2p+1 on 64-127. The two heads' score
    matmuls then occupy disjoint PE row-groups (tile_position auto-derived
    from the base partitions) and can overlap in the array.
  - scores for a (pair, jt) land in one [128, 1024] PSUM tile (head A cols
    0-511 = bank 0, head B cols 512-1023 = bank 1); rel-pos bias (+mask,
    fp8) is folded on the PE via ident8 DoubleRow matmuls; ONE [128, 1024]
    exp per round keeps the ACT per-instruction overhead amortized.
  - softmax denominator: ones-column in V (cp row 64); cp is staged to SBUF
    right after the slab finishes (frees the PSUM bank), 1/denom via DVE
    reciprocal_approx_fast (ACT does nothing but Exp -> no table thrash),
    partition-broadcast via a DRAM bounce, applied by DVE into ctxT.
  - projections / out-projection / norm steps are drip-fed into the stage-1
    rounds through pending-work queues to keep the PE stream dense (HAM).
"""
import os
import sys

if "/opt/trn_rl_repo" not in sys.path:
    sys.path.insert(0, "/opt/trn_rl_repo")

DEBUG = os.environ.get("KDEBUG", "0") == "1"

from contextlib import ExitStack

import ml_dtypes
import numpy as np

B, S, D, H = 2, 2048, 1024, 16
HD = D // H          # 64
NCORES = 8
HPC = 4              # heads per core
NPAIR = 2            # head pairs per core
P = 128
ISLAB = 512
NJT = S // P         # 16
NISL = S // ISLAB    # 4
LAGR = 2             # ctx runs LAGR rounds behind scores
MASK_NEG = np.float32(-200.0)

_CACHE = {}


def _build():
    import concourse.bass as bass
    import concourse.mybir as mybir
    import concourse.tile as tile
    from concourse.tile import add_dep_helper
    from concourse import bacc

    f32 = mybir.dt.float32
    bf16 = mybir.dt.bfloat16
    f8 = mybir.dt.float8e4
    DR = mybir.MatmulPerfMode.DoubleRow
    EXP = mybir.ActivationFunctionType.Exp

    nc = bacc.Bacc(None, target_bir_lowering=False)

    xq = nc.declare_dram_parameter("xq", [8, P, S], bf16, isOutput=False)
    xk = nc.declare_dram_parameter("xk", [8, P, S], bf16, isOutput=False)
    xv = nc.declare_dram_parameter("xv", [8, P, S], bf16, isOutput=False)
    wq = nc.declare_dram_parameter("wq", [8, P, NPAIR, P], bf16, isOutput=False)
    wk = nc.declare_dram_parameter("wk", [8, P, NPAIR, P], bf16, isOutput=False)
    wv = nc.declare_dram_parameter("wv", [8, P, HPC * HD], bf16, isOutput=False)
    wo = nc.declare_dram_parameter("wo", [2, P, D], bf16, isOutput=False)
    bqk = nc.declare_dram_parameter("bqk", [1, 2, NPAIR, P], bf16,
                                    isOutput=False)
    bv_r = nc.declare_dram_parameter("bv_r", [1, HPC * HD], bf16, isOutput=False)
    bias_c = nc.declare_dram_parameter(
        "bias_c", [HPC, NISL, 64, NJT, 2, ISLAB], f8, isOutput=False)
    ident8 = nc.declare_dram_parameter("ident8", [64, 2, P], f8, isOutput=False)
    out_p = nc.declare_dram_parameter("out_p", [S, D], f32, isOutput=True)
    if DEBUG:
        dbg_stg = nc.declare_dram_parameter("dbg_stg", [HD + 1, ISLAB], f32,
                                            isOutput=True)
        dbg_rec = nc.declare_dram_parameter("dbg_rec", [1, ISLAB], f32,
                                            isOutput=True)
        dbg_bsb = nc.declare_dram_parameter("dbg_bsb", [HD, ISLAB], f32,
                                            isOutput=True)
        dbg_ctxT = nc.declare_dram_parameter("dbg_ctxT", [P, 2, S], bf16,
                                             isOutput=True)
        dbg_et = nc.declare_dram_parameter("dbg_et", [P, 2 * ISLAB], bf16,
                                           isOutput=True)

    with tile.TileContext(nc) as tc, ExitStack() as big:
        consts = big.enter_context(tc.tile_pool(name="consts", bufs=1))
        persist = big.enter_context(tc.tile_pool(name="persist", bufs=1))

        ident8_sb = consts.tile([64, 2, P], f8)
        nc.sync.dma_start(ident8_sb, ident8[:])
        ones_row = consts.tile([1, ISLAB], bf16)
        nc.vector.memset(ones_row, 1.0)
        bqk_sb = consts.tile([1, 2, NPAIR, P], bf16)
        nc.sync.dma_start(bqk_sb, bqk[:])
        bv_sb = consts.tile([1, HPC * HD], bf16)
        nc.sync.dma_start(bv_sb, bv_r[:])

        wq_sb = consts.tile([P, 8, NPAIR, P], bf16)
        wk_sb = consts.tile([P, 8, NPAIR, P], bf16)
        wv_sb = consts.tile([P, 8, HPC * HD], bf16)
        wo_sb = consts.tile([P, 2, D], bf16)

        qt2 = persist.tile([P, NPAIR, S], bf16)   # [d(pair-packed), pair, i]
        kt2 = persist.tile([P, NPAIR, S], bf16)   # [d(pair-packed), pair, j]
        v_full = persist.tile([P, NJT, HPC, HD + 1], bf16)
        ctxT = persist.tile([P, 2, S], bf16)      # [(h%2)*64+d, h//2, i]

        ones_col = consts.tile([P, 1], f32)
        nc.vector.memset(ones_col, 1.0)
        for jt in range(NJT):
            nc.vector.tensor_copy(
                v_full[:, jt, :, HD:HD + 1],
                ones_col[:, None, :].to_broadcast((P, HPC, 1)))

        # ---------------- pools -----------------------------------------
        xqp = big.enter_context(tc.tile_pool(name="xqp", bufs=8))
        xvp = big.enter_context(tc.tile_pool(name="xvp", bufs=8))
        sbias = big.enter_context(tc.tile_pool(name="sbias", bufs=4))
        sexp = big.enter_context(tc.tile_pool(name="sexp", bufs=4))
        sstg = big.enter_context(tc.tile_pool(name="sstg", bufs=2))
        snrm = big.enter_context(tc.tile_pool(name="snrm", bufs=4))
        dnrm = big.enter_context(tc.tile_pool(name="dnrm", bufs=2, space="DRAM"))
        so = big.enter_context(tc.tile_pool(name="so", bufs=3))

        sps = big.enter_context(tc.tile_pool(name="sps", bufs=2, space="PSUM"))
        cpa = big.enter_context(tc.tile_pool(name="cpa", bufs=1, space="PSUM"))
        cpb = big.enter_context(tc.tile_pool(name="cpb", bufs=1, space="PSUM"))
        ppp = big.enter_context(tc.tile_pool(name="ppp", bufs=1, space="PSUM"))
        opp = big.enter_context(tc.tile_pool(name="opp", bufs=1, space="PSUM"))

        nc.sync.dma_start(wq_sb, wq[:].rearrange("dk p r m -> p dk r m"))
        nc.sync.dma_start(wk_sb, wk[:].rearrange("dk p r m -> p dk r m"))
        nc.sync.dma_start(wv_sb, wv[:].rearrange("dk p m -> p dk m"))
        nc.sync.dma_start(wo_sb, wo[:].rearrange("kt p n -> p kt n"))

        def load_bias(h, isl, jtg):
            bt = sbias.tile([64, 4, 2, ISLAB], f8, tag="bt", name="bt")
            nc.sync.dma_start(bt, bias_c[h, isl, :, jtg * 4:(jtg + 1) * 4])
            return bt

        bt_cur = {0: load_bias(0, 0, 0), 1: load_bias(1, 0, 0)}
        bt_nxt = {}

        # ---------------- projection helpers ----------------------------
        xk_t, xq_t, xv_t = [None] * 8, [None] * 8, [None] * 8

        def load_x(which, dk, pool=None):
            pool, dram, arr = {
                "k": (pool, xk, xk_t), "q": (xqp, xq, xq_t),
                "v": (xvp, xv, xv_t)}[which]
            t = pool.tile([P, S], bf16, tag="x" + which)
            nc.sync.dma_start(t, dram[dk])
            arr[dk] = t

        def kq_proj(which, pair, sl):
            # one 512-wide slab of K or Q projection for one head pair
            w_sb = wk_sb if which == "k" else wq_sb
            x_t = xk_t if which == "k" else xq_t
            dst = kt2 if which == "k" else qt2
            brow = bqk_sb[0:1, 0 if which == "q" else 1, pair, :]
            pp = ppp.tile([P, ISLAB], f32, tag="pp", name="pp")
            for dk in range(8):
                nc.tensor.matmul(
                    pp, w_sb[:, dk, pair, :],
                    x_t[dk][:, sl * ISLAB:(sl + 1) * ISLAB],
                    start=(dk == 0), stop=False)
            nc.tensor.matmul(pp, brow, ones_row, start=False, stop=True)
            nc.vector.tensor_copy(
                dst[:, pair, sl * ISLAB:(sl + 1) * ISLAB], pp)

        def v_proj(jt):
            pp = ppp.tile([P, ISLAB], f32, tag="pp", name="pp")
            vp = pp[:, 0:HPC * HD]
            for dk in range(8):
                nc.tensor.matmul(
                    vp, xv_t[dk][:, jt * P:(jt + 1) * P], wv_sb[:, dk, :],
                    start=(dk == 0), stop=False)
            nc.tensor.matmul(vp, ones_row[:, 0:P], bv_sb, start=False, stop=True)
            nc.vector.tensor_copy(
                v_full[:, jt, :, :HD],
                vp.rearrange("p (h d) -> p h d", h=HPC))

        # ---------------- deferred-work machinery ------------------------
        pend = []   # FIFO of deferred emissions (projections, outproj)

        def drain(n):
            for _ in range(min(n, len(pend))):
                pend.pop(0)()

        def make_norm_steps(cp_t, h, isl):
            # stage cp (incl. denominator row) to SBUF first -> frees the
            # PSUM bank; then recip / broadcast-bounce / normalize.
            st = {}

            def s_stage():
                st["stg"] = sstg.tile([HD + 1, ISLAB], f32, tag="stg",
                                      name="stg")
                nc.vector.tensor_copy(st["stg"], cp_t)

            def s_dma1():
                # denominator row (partition 64) -> DRAM
                st["dn"] = dnrm.tile([1, ISLAB], f32, tag="dn", name="dn")
                nc.sync.dma_start(st["dn"], st["stg"][HD:HD + 1, :])

            def s_dma2():
                # broadcast-read the denominator into 64 partitions (base 0)
                dn = st["dn"]
                st["bsb"] = snrm.tile([HD, ISLAB], f32, tag="bsb", name="bsb")
                nc.sync.dma_start(st["bsb"], bass.AP(
                    tensor=dn.tensor, offset=dn.offset,
                    ap=[[0, HD]] + list(dn[0].ap)))

            def s_rec():
                # reciprocal_approx_fast mishandles base_partition != 0, so
                # run it after the broadcast where the tile starts at 0.
                st["rb"] = snrm.tile([HD, ISLAB], f32, tag="rb", name="rb")
                nc.vector.reciprocal_approx_fast(
                    out=st["rb"], in_=st["bsb"])

            def s_mul():
                nc.vector.tensor_mul(
                    ctxT[(h % 2) * 64:(h % 2) * 64 + 64, h // 2,
                         isl * ISLAB:(isl + 1) * ISLAB],
                    st["stg"][0:HD, :], st["rb"])
                if DEBUG and h == 0 and isl == 0:
                    nc.sync.dma_start(dbg_stg[:], st["stg"])
                    nc.sync.dma_start(dbg_rec[:], st["rb"][0:1, :])
                    nc.sync.dma_start(dbg_bsb[:], st["bsb"])

            return [s_stage, s_dma1, s_dma2, s_rec, s_mul]

        def make_outproj(isl):
            steps = []
            for it in range(isl * 4, isl * 4 + 4):
                for nh in range(2):
                    def run(it=it, nh=nh):
                        op = opp.tile([P, ISLAB], f32, tag="op", name="op")
                        for kt in range(2):
                            nc.tensor.matmul(
                                op, ctxT[:, kt, it * P:(it + 1) * P],
                                wo_sb[:, kt, nh * ISLAB:(nh + 1) * ISLAB],
                                start=(kt == 0), stop=(kt == 1))
                        ot = so.tile([P, ISLAB], f32, tag="ot", name="ot")
                        nc.vector.tensor_copy(ot, op)
                        nc.sync.dma_start(
                            out_p[it * P:(it + 1) * P,
                                  nh * ISLAB:(nh + 1) * ISLAB], ot)
                    steps.append(run)
            return steps

        # ---------------- prologue --------------------------------------
        with tc.tile_pool(name="xkp", bufs=8) as xkp:
            for dk in range(8):
                load_x("k", dk, pool=xkp)
            for dk in range(8):
                load_x("q", dk)
            for dk in range(8):
                load_x("v", dk)
            # K fully, Q islab 0, V jt 0..3; the rest drip-feeds via pend
            for sl in range(NISL):
                for pair in range(NPAIR):
                    kq_proj("k", pair, sl)
        for pair in range(NPAIR):
            kq_proj("q", pair, 0)
        for jt in range(4):
            v_proj(jt)
        for jt in range(4, NJT):
            pend.append(lambda jt=jt: v_proj(jt))

        # ---------------- stage 1: blocks of (islab, head-pair) ----------
        blocks = [(isl, pair) for isl in range(NISL) for pair in range(NPAIR)]
        carry = []            # ctx tail closures from previous block
        norm_pend = []        # norm steps from previous block
        for bi, (isl, pair) in enumerate(blocks):
            hA, hB = 2 * pair, 2 * pair + 1
            if bi > 0:
                bt_cur = {hA: bt_nxt[hA], hB: bt_nxt[hB]}
            qt_A = qt2[0:64, pair, isl * ISLAB:(isl + 1) * ISLAB]
            qt_B = qt2[64:128, pair, isl * ISLAB:(isl + 1) * ISLAB]
            cpa_t = cpa.tile([HD + 1, ISLAB], f32, tag="cpa", name="cpa")
            cpb_t = cpb.tile([HD + 1, ISLAB], f32, tag="cpb", name="cpb")
            ets = [None] * NJT
            sc = [None] * NJT

            for jt in range(NJT):
                # 1) previous block's ctx tail (must precede its norm steps)
                if carry:
                    carry.pop(0)()
                # 2) previous block's norm pipeline, 2 steps per round
                if jt >= LAGR:
                    for _ in range(2):
                        if norm_pend:
                            norm_pend.pop(0)()
                # 3) drip-feed projections / out-projection
                drain(2)
                # 4) bias chunk rotation + prefetch
                if jt % 4 == 0 and jt > 0:
                    bt_cur = {hA: bt_nxt[hA], hB: bt_nxt[hB]}
                if jt % 4 == 0:
                    if jt < 12:
                        for h_ in (hA, hB):
                            bt_nxt[h_] = load_bias(h_, isl, jt // 4 + 1)
                    elif bi + 1 < len(blocks):
                        isl_n, pair_n = blocks[bi + 1]
                        for h_ in (2 * pair_n, 2 * pair_n + 1):
                            bt_nxt[h_] = load_bias(h_, isl_n, 0)

                # 5) scores: the two heads go to disjoint PE row groups
                sp = sps.tile([P, 2 * ISLAB], f32, tag="sp", name="sp")
                smA = nc.tensor.matmul(
                    sp[:, 0:ISLAB], kt2[0:64, pair, jt * P:(jt + 1) * P],
                    qt_A, start=True, stop=False)
                nc.tensor.matmul(
                    sp[:, ISLAB:2 * ISLAB],
                    kt2[64:128, pair, jt * P:(jt + 1) * P],
                    qt_B, start=True, stop=False)
                sc[jt] = smA
                # 6) fp8 DoubleRow bias folds (shared ident8 stationary)
                nc.tensor.matmul(sp[:, 0:ISLAB], ident8_sb,
                                 bt_cur[hA][:, jt % 4],
                                 start=False, stop=True, perf_mode=DR)
                nc.tensor.matmul(sp[:, ISLAB:2 * ISLAB], ident8_sb,
                                 bt_cur[hB][:, jt % 4],
                                 start=False, stop=True, perf_mode=DR)
                # 7) one wide exp for both heads
                et = sexp.tile([P, 2 * ISLAB], bf16, tag="et", name="et")
                nc.scalar.activation(et, sp, EXP)
                ets[jt] = et
                if DEBUG and bi == 0 and jt == 0:
                    nc.sync.dma_start(dbg_et[:], et)

                # 8) ctx, LAGR rounds behind
                if jt >= LAGR:
                    j2 = jt - LAGR
                    for h_, cp_t, half in ((hA, cpa_t, 0), (hB, cpb_t, 1)):
                        cmm = nc.tensor.matmul(
                            cp_t, v_full[:, j2, h_, :],
                            ets[j2][:, half * ISLAB:(half + 1) * ISLAB],
                            start=(j2 == 0), stop=(j2 == NJT - 1))
                        add_dep_helper(sc[jt].ins, cmm.ins, sync=False,
                                       reason="preserve scores/ctx skew")

            # ctx tail -> start of next block; then norm steps
            carry = []
            for j2 in range(NJT - LAGR, NJT):
                def tail(j2=j2, ets=ets, cpa_t=cpa_t, cpb_t=cpb_t,
                         hA=hA, hB=hB):
                    for h_, cp_t, half in ((hA, cpa_t, 0), (hB, cpb_t, 1)):
                        nc.tensor.matmul(
                            cp_t, v_full[:, j2, h_, :],
                            ets[j2][:, half * ISLAB:(half + 1) * ISLAB],
                            start=(j2 == 0), stop=(j2 == NJT - 1))
                carry.append(tail)
            nsA = make_norm_steps(cpa_t, hA, isl)
            nsB = make_norm_steps(cpb_t, hB, isl)
            norm_pend = [s for pairsteps in zip(nsA, nsB) for s in pairsteps]

            # q projection for the next islab during pair-1 blocks
            if pair == 1 and isl + 1 < NISL:
                for pr in range(NPAIR):
                    pend.append(lambda pr=pr, sl=isl + 1: kq_proj("q", pr, sl))
            # out-projection of islab isl-1 (norms finished a block ago)
            if pair == 1 and isl >= 1:
                pend.extend(make_outproj(isl - 1))

        # epilogue: leftover tails, norms, out-projections
        for run in carry:
            run()
        for step in norm_pend:
            step()
        drain(len(pend))
        for run in make_outproj(NISL - 1):
            run()
        if DEBUG:
            nc.sync.dma_start(dbg_ctxT[:], ctxT)

    nc.compile()
    return nc


def _get_nc():
    if "nc" not in _CACHE:
        _CACHE["nc"] = _build()
    return _CACHE["nc"]


def _ident8():
    ki, ko, m = np.meshgrid(np.arange(64), np.arange(2), np.arange(P),
                            indexing="ij")
    return (m == 2 * ki + ko).astype(ml_dtypes.float8_e4m3)


def _prep_inputs(query, key, value, mask, relative_pos_bias,
                 Wq, bq, Wk, bk, Wv, bv, Wo, bo):
    f32 = np.float32
    bf = ml_dtypes.bfloat16
    f8 = ml_dtypes.float8_e4m3
    query = np.asarray(query, f32)
    key = np.asarray(key, f32)
    value = np.asarray(value, f32)
    rpb_T = np.ascontiguousarray(
        np.asarray(relative_pos_bias, f32).transpose(2, 0, 1))  # [H, j, i]
    mask_ji = np.asarray(mask)[:, 0].transpose(0, 2, 1)
    madd = np.where(mask_ji == 0, MASK_NEG, f32(0.0)).astype(f32)

    scale = f32(1.0 / np.sqrt(HD))
    Wq_s = np.asarray(Wq, f32) * scale
    bq_s = np.asarray(bq, f32) * scale
    Wk_f = np.asarray(Wk, f32)
    Wv_f = np.asarray(Wv, f32)
    Wo_f = np.asarray(Wo, f32)
    bk_f = np.asarray(bk, f32)
    bv_f = np.asarray(bv, f32)

    def xT(x):   # [S, D] -> [8, 128, S]
        return np.ascontiguousarray(x.T.reshape(8, P, S)).astype(bf)

    in_maps = []
    for c in range(NCORES):
        b = c // 4
        h0 = (c % 4) * HPC
        cols = slice(h0 * HD, (h0 + HPC) * HD)
        bias_hji = rpb_T[h0:h0 + HPC] + madd[b][None]   # [4, j, i]
        # [h, isl, ki, jt, ko, ic]; j = jt*128 + ki*2 + ko, i = isl*512 + ic
        bc = bias_hji.reshape(HPC, NJT, 64, 2, NISL, ISLAB)
        bc = np.ascontiguousarray(bc.transpose(0, 4, 2, 1, 3, 5)).astype(f8)
        in_maps.append({
            "xq": xT(query[b]),
            "xk": xT(key[b]),
            "xv": xT(value[b]),
            "wq": np.ascontiguousarray(
                Wq_s[:, cols].reshape(8, P, NPAIR, P)).astype(bf),
            "wk": np.ascontiguousarray(
                Wk_f[:, cols].reshape(8, P, NPAIR, P)).astype(bf),
            "wv": np.ascontiguousarray(
                Wv_f[:, cols].reshape(8, P, HPC * HD)).astype(bf),
            "wo": np.ascontiguousarray(
                Wo_f[cols, :].reshape(2, P, D)).astype(bf),
            "bqk": np.stack([bq_s[cols], bk_f[cols]]).reshape(
                1, 2, NPAIR, P).astype(bf),
            "bv_r": bv_f[cols].reshape(1, HPC * HD).astype(bf),
            "bias_c": bc,
            "ident8": _ident8(),
        })
    return in_maps


def run_sharded(run_kwargs=None, **inputs):
    """Build + run on 8 cores; returns (output, BassKernelResults)."""
    from concourse.bass_utils import run_bass_kernel_spmd

    nc = _get_nc()
    in_maps = _prep_inputs(**inputs)
    res = run_bass_kernel_spmd(nc, in_maps, list(range(NCORES)),
                               **(run_kwargs or {}))
    bo = np.asarray(inputs["bo"], np.float32)
    out = np.zeros((B, S, D), np.float32)
    for c in range(NCORES):
        out[c // 4] += res.results[c]["out_p"]
    out += bo[None, None, :]
    return out, res


def kernel(**inputs):
    out, _ = run_sharded(**inputs)
    return out
